# revision 1
# baseline (speedup 1.0000x reference)
"""Trainium2 Bass kernel for a 2-layer GCN + FC head (nn_CNNGNNModel).

Reference computation (PyG GCNConv semantics, symmetric normalization with
self-loops):
    deg[i]  = in-degree(i) + 1 ;  dinv = deg^-0.5
    A_hat   = D^-1/2 (A + I) D^-1/2   (aggregation by destination)
    h1 = relu(A_hat @ (x @ W1) + b1)
    h2 = relu(A_hat @ (h1 @ W2) + b2)
    out = h2 @ Wfc + bfc

The per-edge weight dinv[src]*dinv[dst] is separable: source-side dinv is
folded into the feature rows on the way out of each matmul; dest-side dinv is
folded into the one-hot aggregation matrices.

Distribution (8 NeuronCores, SPMD single program):
  - Nodes are assigned host-side to table positions pos in [0, 100352); core
    c owns positions [c*12544, (c+1)*12544) both as destinations (12544 =
    98 blocks of 128 dest slots) and as its shard of the source-feature
    table.  Positions are chosen to balance per-(block, source-window) edge
    counts (greedy window coloring + degree-snake block packing).
  - Each layer: local matmul of the core's node block -> dinv-scaled bf16
    features -> AllGather to a full table [100352, 256] bf16 -> per dest
    block, gather incoming source rows in rounds of 128 edges (dma_gather,
    <=1024 indices per call; the int16 index limit splits the table into 5
    windows of <=20096 rows addressed via the in_ AP base offset).
  - Aggregation is done on the Tensor engine: per round, a [128 edge x 128
    dest] one-hot matrix M (M[e, d] = dinv_dst[e] if dest(e)==d) is built on
    DVE via tensor_scalar(is_equal, mult) against an iota constant, then
    aggT[f, d] += G[e, f]^T M accumulates in PSUM, transposed so the result
    feeds the next matmul's lhsT directly (no transposes anywhere).
  - Weights are replicated; output [12544, 1000] bf16 per core is
    reassembled (inverse position map) and upcast to f32 on the host.
"""

import numpy as np
import ml_dtypes

import concourse.bass as bass
import concourse.bacc as bacc
import concourse.mybir as mybir
import concourse.tile as tile
from concourse.bass_utils import run_bass_kernel_spmd

BF16 = ml_dtypes.bfloat16

N_CORES = 8
N_NODES = 100000
IN_DIM = 512
HID = 256
NCLS = 1000
P = 128
SLOTS = 12544            # positions per core (98 blocks of 128)
BLOCKS = SLOTS // P      # 98
NPOS = N_CORES * SLOTS   # 100352 table rows
NW = 5                   # gather windows (int16 index limit)
W_BLOCKS = [157, 157, 157, 157, 156]          # blocks per window
WB = np.concatenate([[0], np.cumsum(W_BLOCKS)])  # window block boundaries
W_ROWS0 = WB * P                              # window row bases
MAX_CALL_ROUNDS = 8      # 8*128 = 1024 idx per dma_gather call (HW limit)


def _wrap_idx(flat_idx: np.ndarray) -> np.ndarray:
    """Wrap a flat int16 index array [n] (n % 16 == 0) into the dma_gather
    SBUF layout [128, n//16]: position j -> (partition j%16, column j//16),
    replicated across the eight 16-partition bands."""
    n = flat_idx.shape[0]
    band = flat_idx.reshape(n // 16, 16).T  # [16, n//16]
    return np.tile(band, (8, 1)).astype(np.int16)


# per-(block, source-window) edge-count target: 7 gather rounds (<=896
# edges) for every window; 35 rounds of 128 = 4480 covers the ~4209 mean
# block degree with enough slack that the repair pass can push every
# (block, window) under the cap, making the shared round plan exactly
# 7 rounds per (block, window).
W_TARGET = np.array([896, 896, 896, 896, 896], np.float32)


def _assign_positions(row, col, deg, rng):
    """Color nodes into the 5 source windows (balancing each destination's
    in-edge window spread against W_TARGET), then pack each window's nodes
    into its dest blocks with a worst-dimension-first constructive pass.
    Returns pos[node]."""
    w_slots = np.array([b * P for b in W_BLOCKS])
    caps = w_slots - np.array([70, 70, 70, 70, 72])
    frac = w_slots / float(NPOS)

    # out-edge CSR (self-loops included: node i has an out-edge to itself)
    all_src = np.concatenate([row, np.arange(N_NODES)])
    all_dst = np.concatenate([col, np.arange(N_NODES)])
    o = np.argsort(all_src, kind="stable")
    sr = all_src[o]
    sc = all_dst[o]
    starts = np.searchsorted(sr, np.arange(N_NODES + 1))
    deg_out = np.diff(starts)
    target = deg.astype(np.float32)[:, None] * frac[None, :].astype(np.float32)

    color = np.full(N_NODES, -1, np.int8)
    kmat = np.zeros((N_NODES, NW), np.int32)
    sizes = np.zeros(NW, np.int64)
    # heavy sources first: they get spread by target deviation while all
    # windows are open; the light tail then fills whatever capacity (mostly
    # the low-target window 4) remains.
    order_src = rng.permutation(N_NODES)
    order_src = order_src[np.argsort(-deg_out[order_src], kind="stable")]
    B = 1000
    for i in range(0, N_NODES, B):
        batch = order_src[i:i + B]
        reps = deg_out[batch]
        idx = np.concatenate(
            [np.arange(starts[s], starts[s + 1]) for s in batch]
        )
        dsts = sc[idx]
        srcrep = np.repeat(np.arange(len(batch)), reps)
        dev = kmat[dsts].astype(np.float32) - target[dsts]
        score = np.zeros((len(batch), NW), np.float32)
        np.add.at(score, srcrep, dev)
        score += (sizes / caps).astype(np.float32) * 0.5 * reps[:, None]
        score[:, sizes >= caps] = 1e18
        ch = score.argmin(1).astype(np.int8)
        color[batch] = ch
        np.add.at(sizes, ch, 1)
        np.add.at(kmat, (dsts, ch[srcrep]), 1)

    # hard-clamp: batched greedy can overshoot caps; move the lightest
    # nodes out of overfull windows into the ones with most slack
    for w in range(NW):
        excess = int(sizes[w] - caps[w])
        if excess <= 0:
            continue
        cand = np.where(color == w)[0]
        cand = cand[np.argsort(deg_out[cand], kind="stable")][:excess]
        for n in cand:
            w2 = int(np.argmax(caps - sizes))
            color[n] = w2
            sizes[w] -= 1
            sizes[w2] += 1
            dsts_n = sc[starts[n]:starts[n + 1]]
            np.add.at(kmat, (dsts_n, w), -1)
            np.add.at(kmat, (dsts_n, w2), 1)

    pos = np.empty(N_NODES, np.int64)
    tgt = W_TARGET[None, :]
    for w in range(NW):
        nodes_w = np.where(color == w)[0]
        kk = kmat[nodes_w].astype(np.float32)
        nb = W_BLOCKS[w]
        # worst-dimension-first constructive packing: place heavy nodes
        # first, each into the block whose post-insert worst dimension
        # (relative to the per-window round targets) is smallest.
        order_n = np.argsort(-(kk / tgt).max(1), kind="stable")
        cnt_b = np.zeros((nb, NW), np.float32)
        nslot = np.zeros(nb, np.int64)
        blk = np.empty(len(nodes_w), np.int64)
        slot_of = np.empty(len(nodes_w), np.int64)
        for j in order_n:
            score = ((cnt_b + kk[j][None, :]) / tgt).max(1)
            score[nslot >= P] = 1e9
            b = int(np.argmin(score))
            blk[j] = b
            slot_of[j] = nslot[b]
            nslot[b] += 1
            cnt_b[b] += kk[j]
        # repair: swap nodes between blocks until no (block, dim) exceeds
        # its round target (or no improving swap exists)
        T = W_TARGET
        for _pass in range(8):
            viol = np.argwhere(cnt_b > T[None, :])
            if len(viol) == 0:
                break
            improved = False
            for b, wd in viol:
                if cnt_b[b, wd] <= T[wd]:
                    continue
                members = np.where(blk == b)[0]
                dn = members[np.argsort(-kk[members, wd], kind="stable")][:6]
                rb = np.argsort(cnt_b[:, wd], kind="stable")[:12]
                done_bw = False
                for b2 in rb:
                    if b2 == b or done_bw:
                        continue
                    m2 = np.where(blk == b2)[0]
                    up = m2[np.argsort(kk[m2, wd], kind="stable")][:6]
                    for n1 in dn:
                        k1 = kk[n1]
                        ok = False
                        for n2 in up:
                            k2 = kk[n2]
                            if k2[wd] >= k1[wd]:
                                break
                            nb_ = cnt_b[b] - k1 + k2
                            nb2 = cnt_b[b2] + k1 - k2
                            if (nb_ <= np.maximum(cnt_b[b], T)).all() and (
                                    nb2 <= T).all():
                                blk[n1], blk[n2] = b2, b
                                slot_of[n1], slot_of[n2] = slot_of[n2], slot_of[n1]
                                cnt_b[b] = nb_
                                cnt_b[b2] = nb2
                                improved = True
                                ok = True
                                break
                        if ok and cnt_b[b, wd] <= T[wd]:
                            done_bw = True
                            break
            if not improved:
                break
        pos[nodes_w] = (WB[w] + blk) * P + slot_of
    return pos


def _preprocess(x, edge_index, W1, b1, W2, b2, Wfc, bfc):
    """Host-side graph preprocessing. Returns (plan, in_maps, pos)."""
    row = np.asarray(edge_index[0], dtype=np.int64)
    col = np.asarray(edge_index[1], dtype=np.int64)

    deg = np.bincount(col, minlength=N_NODES).astype(np.int64) + 1
    dinv = (1.0 / np.sqrt(deg.astype(np.float32))).astype(np.float32)

    rng = np.random.default_rng(12345)
    pos = _assign_positions(row, col, deg, rng)

    # zero (pad) rows per window: highest snake rank of each window is free
    occupied = np.zeros(NPOS, bool)
    occupied[pos] = True
    zrow_local = np.empty(NW, np.int64)
    for w in range(NW):
        free = np.where(~occupied[W_ROWS0[w]:W_ROWS0[w + 1]])[0]
        assert len(free) > 0
        zrow_local[w] = free[-1]

    # --- edge lists (self-loops included) in position space
    all_src = np.concatenate([row, np.arange(N_NODES)])
    all_dst = np.concatenate([col, np.arange(N_NODES)])
    spos = pos[all_src]
    dpos = pos[all_dst]
    w_e = np.searchsorted(W_ROWS0[1:], spos, side="right")
    lidx = (spos - W_ROWS0[w_e]).astype(np.int64)
    gb = dpos // P
    slot = (dpos % P).astype(np.float32)
    core_e = gb // BLOCKS
    lb_e = gb % BLOCKS
    dinv_e = dinv[all_dst]

    # counts per (core, lb, w) and the shared round plan
    key_full = (core_e * BLOCKS + lb_e) * NW + w_e
    cnt = np.bincount(key_full, minlength=N_CORES * BLOCKS * NW).reshape(
        N_CORES, BLOCKS, NW
    )
    Rbw = -(-cnt.max(axis=0) // P)  # [BLOCKS, NW] rounds (ceil)
    R_total = int(Rbw.sum())
    real_edges = int(cnt.sum())
    inflation = R_total * P * N_CORES / real_edges

    # call plan: per (lb, w) chunks of <= MAX_CALL_ROUNDS rounds
    # entries: (lb, w, r0_global, rounds, idx8_off)
    rbase = np.zeros((BLOCKS, NW), np.int64)
    np.cumsum(Rbw.ravel()[:-1], out=rbase.ravel()[1:])
    calls = []
    blk_calls = [[] for _ in range(BLOCKS)]
    idx8_off = 0
    blk_idx8 = np.zeros((BLOCKS, 2), np.int64)  # per-block idx8 [start, len]
    for lb in range(BLOCKS):
        blk_idx8[lb, 0] = idx8_off
        for w in range(NW):
            R = int(Rbw[lb, w])
            r0 = int(rbase[lb, w])
            taken = 0
            while taken < R:
                ch = min(MAX_CALL_ROUNDS, R - taken)
                c_rec = (lb, w, r0 + taken, ch, idx8_off)
                calls.append(c_rec)
                blk_calls[lb].append(c_rec)
                idx8_off += ch * 8
                taken += ch
        blk_idx8[lb, 1] = idx8_off - blk_idx8[lb, 0]
    idx8_total = idx8_off

    # --- per-core gather metadata
    order = np.lexsort((w_e, lb_e, core_e))
    so_core = core_e[order]
    so_key = (lb_e[order] * NW + w_e[order]).astype(np.int64)
    so_lidx = lidx[order]
    so_slot = slot[order]
    so_dinv = dinv_e[order]
    core_starts = np.searchsorted(so_core, np.arange(N_CORES + 1))

    rbase_flat = rbase.ravel()
    Rbw_flat = Rbw.ravel()
    in_maps = []
    xb = np.ascontiguousarray(x).astype(BF16)
    w1_in = np.ascontiguousarray(
        np.asarray(W1).astype(BF16).reshape(4, P, HID).transpose(1, 0, 2).reshape(P, 4 * HID)
    )
    w2_in = np.ascontiguousarray(
        np.asarray(W2).astype(BF16).reshape(2, P, HID).transpose(1, 0, 2).reshape(P, 2 * HID)
    )
    wfc_in = np.ascontiguousarray(
        np.asarray(Wfc).astype(BF16).reshape(2, P, NCLS).transpose(1, 0, 2).reshape(P, 2 * NCLS)
    )
    iota_in = np.tile(np.arange(P, dtype=np.float32).astype(BF16)[None, :], (P, 1))
    has_b1 = bool(np.any(b1)); has_b2 = bool(np.any(b2)); has_bfc = bool(np.any(bfc))
    b1_in = np.asarray(b1, np.float32).reshape(2, P).T.copy()   # [128, 2]
    b2_in = np.asarray(b2, np.float32).reshape(2, P).T.copy()
    bfc_in = np.tile(np.asarray(bfc, np.float32).astype(BF16)[None, :], (P, 1))

    group_zrow = np.repeat(zrow_local[None, :], BLOCKS, axis=0).ravel()

    for c in range(N_CORES):
        s0, s1 = core_starts[c], core_starts[c + 1]
        keys_c = so_key[s0:s1]
        # j-within-group for each edge (groups are contiguous, sorted)
        grp_start = np.searchsorted(keys_c, np.arange(BLOCKS * NW))
        jw = np.arange(s1 - s0) - grp_start[keys_c]
        ecol = rbase_flat[keys_c] + jw // P
        erow = jw % P

        destid = np.zeros((P, R_total), np.float32)
        dinvdst = np.ones((P, R_total), np.float32)
        destid[erow, ecol] = so_slot[s0:s1]
        dinvdst[erow, ecol] = so_dinv[s0:s1]

        # flat (round-major) local indices, defaulting to the window's 0-row
        flat_idx = np.repeat(group_zrow, Rbw_flat * P).astype(np.int64)
        flat_idx[ecol * P + erow] = so_lidx[s0:s1]
        assert flat_idx.max() < 32768
        idxs2d = np.empty((P, idx8_total), np.int16)
        for (lb, w, r0, ch, i8) in calls:
            seg = flat_idx[r0 * P:(r0 + ch) * P].astype(np.int16)
            idxs2d[:, i8:i8 + ch * 8] = _wrap_idx(seg)

        # dense per-core inputs
        ids_c = np.where((pos >= c * SLOTS) & (pos < (c + 1) * SLOTS))[0]
        lpos = pos[ids_c] - c * SLOTS
        A = np.zeros((SLOTS, IN_DIM), BF16)
        A[lpos] = xb[ids_c]
        xtt = np.ascontiguousarray(
            A.reshape(BLOCKS, P, 4, P).transpose(0, 3, 2, 1).reshape(BLOCKS, P, IN_DIM)
        )
        dv = np.ones(SLOTS, np.float32)
        dv[lpos] = dinv[ids_c]
        dvp = np.ascontiguousarray(dv.reshape(BLOCKS, P).T)  # [128, 98]

        m = {
            "xtt": xtt,
            "dinvp": dvp,
            "idxs": idxs2d,
            "destid": destid,
            "dinvdst": dinvdst,
            "iota": iota_in,
            "w1": w1_in,
            "w2": w2_in,
            "wfc": wfc_in,
        }
        if has_b1:
            m["b1h"] = b1_in
        if has_b2:
            m["b2h"] = b2_in
        if has_bfc:
            m["bfcb"] = bfc_in
        in_maps.append(m)

    plan = {
        "blk_calls": blk_calls,
        "blk_idx8": blk_idx8.tolist(),
        "rbase": rbase.tolist(),
        "Rbw": Rbw.tolist(),
        "R_total": R_total,
        "idx8_total": idx8_total,
        "n_calls": len(calls),
        "has_b1": has_b1,
        "has_b2": has_b2,
        "has_bfc": has_bfc,
        "inflation": inflation,
    }
    return plan, in_maps, pos


def _build_program(plan, sim_single_core=False, stop_after="full"):
    """Build the SPMD Bass program (one program, all cores).

    stop_after: one of "mm1", "ag1", "g1", "ag2", "g2", "full" — truncates
    the program after that phase (for bisection/debug)."""
    STAGES = ["mm1", "ag1", "g1", "ag2", "g2", "full"]
    stop_idx = STAGES.index(stop_after)
    nc = bacc.Bacc("TRN2", target_bir_lowering=False, debug=False,
                   num_devices=N_CORES)
    dt = mybir.dt

    R_total = plan["R_total"]
    idx8_total = plan["idx8_total"]
    blk_calls = plan["blk_calls"]
    blk_idx8 = plan["blk_idx8"]
    rbase = plan["rbase"]
    Rbw = plan["Rbw"]

    xtt = nc.dram_tensor("xtt", [BLOCKS, P, IN_DIM], dt.bfloat16, kind="ExternalInput")
    dinvp = nc.dram_tensor("dinvp", [P, BLOCKS], dt.float32, kind="ExternalInput")
    idxs = nc.dram_tensor("idxs", [P, idx8_total], dt.int16, kind="ExternalInput")
    destid = nc.dram_tensor("destid", [P, R_total], dt.float32, kind="ExternalInput")
    dinvdst = nc.dram_tensor("dinvdst", [P, R_total], dt.float32, kind="ExternalInput")
    iota = nc.dram_tensor("iota", [P, P], dt.bfloat16, kind="ExternalInput")
    w1 = nc.dram_tensor("w1", [P, 4 * HID], dt.bfloat16, kind="ExternalInput")
    w2 = nc.dram_tensor("w2", [P, 2 * HID], dt.bfloat16, kind="ExternalInput")
    wfc = nc.dram_tensor("wfc", [P, 2 * NCLS], dt.bfloat16, kind="ExternalInput")
    b1h = (nc.dram_tensor("b1h", [P, 2], dt.float32, kind="ExternalInput")
           if plan["has_b1"] else None)
    b2h = (nc.dram_tensor("b2h", [P, 2], dt.float32, kind="ExternalInput")
           if plan["has_b2"] else None)
    bfcb = (nc.dram_tensor("bfcb", [P, NCLS], dt.bfloat16, kind="ExternalInput")
            if plan["has_bfc"] else None)
    out = nc.dram_tensor("out", [SLOTS, NCLS], dt.bfloat16, kind="ExternalOutput")

    hloc1 = nc.dram_tensor("hloc1", [SLOTS, HID], dt.bfloat16)
    hloc2 = nc.dram_tensor("hloc2", [SLOTS, HID], dt.bfloat16)
    hfull1 = nc.dram_tensor("hfull1", [NPOS, HID], dt.bfloat16, addr_space="Shared")
    hfull2 = nc.dram_tensor("hfull2", [NPOS, HID], dt.bfloat16, addr_space="Shared")

    RELU = mybir.ActivationFunctionType.Relu
    COPY = mybir.ActivationFunctionType.Copy

    with tile.TileContext(nc) as tc:
        with (
            tc.tile_pool(name="const", bufs=1) as constp,
            tc.tile_pool(name="xt", bufs=3) as xtp,
            tc.tile_pool(name="hl", bufs=4) as hlp,
            tc.tile_pool(name="idx", bufs=3) as idxp,
            tc.tile_pool(name="g", bufs=10) as gp,
            tc.tile_pool(name="m", bufs=8) as mp,
            tc.tile_pool(name="at", bufs=4) as atp,
            tc.tile_pool(name="fco", bufs=2) as fcop,
            tc.tile_pool(name="mmps", bufs=2, space="PSUM") as mmps,
            tc.tile_pool(name="aggps", bufs=2, space="PSUM") as aggps,
            tc.tile_pool(name="fcps", bufs=1, space="PSUM") as fcps,
        ):
            # resident constants
            w1_sb = constp.tile([P, 4 * HID], dt.bfloat16)
            nc.sync.dma_start(out=w1_sb[:], in_=w1[:])
            w2_sb = constp.tile([P, 2 * HID], dt.bfloat16)
            nc.sync.dma_start(out=w2_sb[:], in_=w2[:])
            wfc_sb = constp.tile([P, 2 * NCLS], dt.bfloat16)
            nc.sync.dma_start(out=wfc_sb[:], in_=wfc[:])
            dv_sb = constp.tile([P, BLOCKS], dt.float32)
            nc.sync.dma_start(out=dv_sb[:], in_=dinvp[:])
            iota_sb = constp.tile([P, P], dt.bfloat16)
            nc.sync.dma_start(out=iota_sb[:], in_=iota[:])
            dst_sb = constp.tile([P, R_total], dt.float32)
            nc.sync.dma_start(out=dst_sb[:], in_=destid[:])
            dnv_sb = constp.tile([P, R_total], dt.float32)
            nc.sync.dma_start(out=dnv_sb[:], in_=dinvdst[:])
            b1_sb = b2_sb = bfc_sb = None
            if b1h is not None:
                b1_sb = constp.tile([P, 2], dt.float32)
                nc.sync.dma_start(out=b1_sb[:], in_=b1h[:])
            if b2h is not None:
                b2_sb = constp.tile([P, 2], dt.float32)
                nc.sync.dma_start(out=b2_sb[:], in_=b2h[:])
            if bfcb is not None:
                bfc_sb = constp.tile([P, NCLS], dt.bfloat16)
                nc.sync.dma_start(out=bfc_sb[:], in_=bfcb[:])

            def all_gather(hloc, hfull):
                if sim_single_core:
                    nc.sync.dma_start(out=hfull[0:SLOTS, :], in_=hloc[:])
                else:
                    nc.gpsimd.collective_compute(
                        "AllGather",
                        mybir.AluOpType.bypass,
                        replica_groups=[list(range(N_CORES))],
                        ins=[hloc[:]],
                        outs=[hfull[:]],
                    )

            def agg_block(lb, hfull, b_sb):
                """Gather + one-hot matmul aggregation for dest block lb.
                Returns aggT sbuf tile [128, 256] bf16 (partition = feat%128,
                cols 0:128 = feats 0..127, cols 128:256 = feats 128..255,
                free position = dest slot), after relu."""
                i8s, i8n = blk_idx8[lb]
                it = idxp.tile([P, i8n], dt.int16, tag="idx")
                nc.sync.dma_start(out=it[:], in_=idxs[:, i8s:i8s + i8n])
                ps0 = aggps.tile([P, P], dt.float32, space="PSUM", tag="agg0")
                ps1 = aggps.tile([P, P], dt.float32, space="PSUM", tag="agg1")
                n_rounds = sum(Rbw[lb])
                gts = []
                for (lb_, w, r0, ch, i8) in blk_calls[lb]:
                    g = gp.tile([P, MAX_CALL_ROUNDS * HID], dt.bfloat16, tag="g")
                    nidx = ch * P
                    nc.gpsimd.dma_gather(
                        g[:, :ch * HID].rearrange("p (l d) -> p l d", d=HID),
                        hfull[W_ROWS0[w]:W_ROWS0[w + 1], :],
                        it[:, i8 - i8s:i8 - i8s + ch * 8],
                        nidx,
                        nidx,
                        HID,
                    )
                    gts.append((g, r0, ch))
                done = 0
                for (g, r0, ch) in gts:
                    for r in range(ch):
                        rc = r0 + r
                        M = mp.tile([P, P], dt.bfloat16, tag="m")
                        nc.vector.tensor_scalar(
                            out=M[:], in0=iota_sb[:],
                            scalar1=dst_sb[:, rc:rc + 1],
                            scalar2=dnv_sb[:, rc:rc + 1],
                            op0=mybir.AluOpType.is_equal,
                            op1=mybir.AluOpType.mult,
                        )
                        for h in range(2):
                            nc.tensor.matmul(
                                out=(ps0 if h == 0 else ps1)[:],
                                lhsT=g[:, r * HID + h * P: r * HID + h * P + P],
                                rhs=M[:],
                                start=(done == 0),
                                stop=(done == n_rounds - 1),
                            )
                        done += 1
                at = atp.tile([P, HID], dt.bfloat16, tag="at")
                for h in range(2):
                    nc.scalar.activation(
                        out=at[:, h * P:(h + 1) * P],
                        in_=(ps0 if h == 0 else ps1)[:],
                        func=RELU,
                        bias=(b_sb[:, h:h + 1] if b_sb is not None else 0.0),
                    )
                return at

            # ---- layer 1: h1' = dinv * (x @ W1)
            for mb in range(BLOCKS):
                at = xtp.tile([P, IN_DIM], dt.bfloat16, tag="xt")
                nc.sync.dma_start(out=at[:], in_=xtt[mb])
                ps = mmps.tile([P, HID], dt.float32, space="PSUM", tag="mm")
                for k in range(4):
                    nc.tensor.matmul(
                        out=ps[:],
                        lhsT=at[:, k * P:(k + 1) * P],
                        rhs=w1_sb[:, k * HID:(k + 1) * HID],
                        start=(k == 0),
                        stop=(k == 3),
                    )
                hl = hlp.tile([P, HID], dt.bfloat16, tag="hl")
                nc.scalar.activation(
                    out=hl[:], in_=ps[:], func=COPY, scale=dv_sb[:, mb:mb + 1],
                )
                nc.scalar.dma_start(out=hloc1[mb * P:(mb + 1) * P, :], in_=hl[:])
            if stop_idx >= 1:
                all_gather(hloc1, hfull1)
            # ---- layer 1 aggregation + layer 2 matmul, fused per block
            for lb in range(BLOCKS if stop_idx >= 2 else 0):
                at1 = agg_block(lb, hfull1, b1_sb)
                ps2 = mmps.tile([P, HID], dt.float32, space="PSUM", tag="mm")
                for k in range(2):
                    nc.tensor.matmul(
                        out=ps2[:],
                        lhsT=at1[:, k * P:(k + 1) * P],
                        rhs=w2_sb[:, k * HID:(k + 1) * HID],
                        start=(k == 0),
                        stop=(k == 1),
                    )
                hl2 = hlp.tile([P, HID], dt.bfloat16, tag="hl")
                nc.scalar.activation(
                    out=hl2[:], in_=ps2[:], func=COPY, scale=dv_sb[:, lb:lb + 1],
                )
                nc.scalar.dma_start(out=hloc2[lb * P:(lb + 1) * P, :], in_=hl2[:])
            if stop_idx >= 3:
                all_gather(hloc2, hfull2)
            # ---- layer 2 aggregation + FC head, fused per block
            for lb in range(BLOCKS if stop_idx >= 4 else 0):
                at2 = agg_block(lb, hfull2, b2_sb)
                fo = fcop.tile([P, NCLS], dt.bfloat16, tag="fco")
                for n in range(2):
                    ps = fcps.tile([P, NCLS // 2], dt.float32, space="PSUM",
                                   tag=f"fc{n}")
                    for k in range(2):
                        nc.tensor.matmul(
                            out=ps[:],
                            lhsT=at2[:, k * P:(k + 1) * P],
                            rhs=wfc_sb[:, k * NCLS + n * (NCLS // 2):
                                       k * NCLS + (n + 1) * (NCLS // 2)],
                            start=(k == 0),
                            stop=(k == 1),
                        )
                    nc.scalar.activation(
                        out=fo[:, n * (NCLS // 2):(n + 1) * (NCLS // 2)],
                        in_=ps[:], func=COPY,
                    )
                if bfc_sb is not None:
                    nc.vector.tensor_tensor(
                        out=fo[:], in0=fo[:], in1=bfc_sb[:],
                        op=mybir.AluOpType.add,
                    )
                if stop_idx >= 5:
                    nc.scalar.dma_start(out=out[lb * P:(lb + 1) * P, :], in_=fo[:])

    nc.compile()
    return nc


def kernel(x, edge_index, W1, b1, W2, b2, Wfc, bfc):
    x = np.asarray(x)
    plan, in_maps, pos = _preprocess(x, edge_index, W1, b1, W2, b2, Wfc, bfc)
    nc = _build_program(plan)
    res = run_bass_kernel_spmd(nc, in_maps, core_ids=list(range(N_CORES)))
    full = np.empty((N_NODES, NCLS), np.float32)
    core = pos // SLOTS
    lrow = pos % SLOTS
    for c in range(N_CORES):
        sel = core == c
        full[sel] = res.results[c]["out"][lrow[sel]].astype(np.float32)
    return full



# revision 4
# speedup vs baseline: 1.0321x; 1.0321x over previous
"""Trainium2 Bass kernel for a 2-layer GCN + FC head (nn_CNNGNNModel) — v2.

Reference computation (PyG GCNConv semantics, symmetric normalization with
self-loops):
    deg[i]  = in-degree(i) + 1 ;  dinv = deg^-0.5
    A_hat   = D^-1/2 (A + I) D^-1/2   (aggregation by destination)
    h1 = relu(A_hat @ (x @ W1) + b1)
    h2 = relu(A_hat @ (h1 @ W2) + b2)
    out = h2 @ Wfc + bfc

The per-edge weight dinv[src]*dinv[dst] is separable: source-side dinv is
folded into the feature rows on the way out of each matmul; dest-side dinv is
folded into the one-hot aggregation matrices.

Distribution (8 NeuronCores, SPMD single program):
  - Nodes are assigned host-side to table positions pos in [0, 100352); core
    c owns positions [c*12544, (c+1)*12544) both as destinations (12544 =
    98 blocks of 128 dest slots) and as its shard of the source-feature
    table.  Positions are chosen to balance per-(block, source-window) edge
    counts (greedy window coloring + degree-aware block packing) to exactly
    [9,8,8,8] gather rounds per block.
  - Each layer: local matmul of the core's node block -> dinv-scaled bf16
    features -> AllGather to a full table [100352, 256] bf16 -> per dest
    block, one dma_gather call per source window (4 windows of 25088 rows;
    int16 indexing allows spans < 32768, <=1152 indices per call) fetching
    the block's incoming rows in rounds of 128 edges.
  - Aggregation on the Tensor engine: per round, a [128 edge x 128 dest]
    one-hot matrix M (M[e, d] = dinv_dst[e] if dest(e)==d) is built on DVE,
    then aggT[f, d] += G[e, f]^T M accumulates in PSUM, transposed so the
    result feeds the next matmul's lhsT directly.
  - The reference's appended self-loops bypass the gather: each block's own
    feature tile (kept resident in SBUF in one big [128, 98*256] buffer) is
    accumulated via a diagonal matrix D[s, d] = dinv[s]*(s==d) built on DVE,
    saving a gather round per block.
  - Weights are replicated; output [12544, 1000] bf16 per core is
    reassembled (inverse position map) and upcast to f32 on the host.
"""

import numpy as np
import ml_dtypes

import concourse.bass as bass
import concourse.bacc as bacc
import concourse.mybir as mybir
import concourse.tile as tile
from concourse.bass_utils import run_bass_kernel_spmd

BF16 = ml_dtypes.bfloat16

N_CORES = 8
N_NODES = 100000
IN_DIM = 512
HID = 256
NCLS = 1000
P = 128
SLOTS = 12544            # positions per core (98 blocks of 128)
BLOCKS = SLOTS // P      # 98
NPOS = N_CORES * SLOTS   # 100352 table rows
NW = 4                   # source windows (int16 index limit: spans < 32768)
W_SPAN = NPOS // NW      # 25088 rows per window
WB_GLOBAL = 196          # global blocks per window
MAX_CALL_ROUNDS = 8      # 1024 idx per dma_gather call (HW limit)
IDXCALLS = 16            # gather calls per idx-chunk load
HLOC_BATCH = 7           # blocks per hloc write DMA
# per-(block, window) edge-count caps guiding the balancer: [9,8,8,8] rounds
W_CAPS = np.array([1152, 1024, 1024, 1024], np.float32)


def _wrap_idx(flat_idx: np.ndarray) -> np.ndarray:
    """Wrap a flat int16 index array [n] (n % 16 == 0) into the dma_gather
    SBUF layout [128, n//16]: position j -> (partition j%16, column j//16),
    replicated across the eight 16-partition bands."""
    n = flat_idx.shape[0]
    band = flat_idx.reshape(n // 16, 16).T  # [16, n//16]
    return np.tile(band, (8, 1)).astype(np.int16)


def _assign_positions(row, col, rng):
    """Color nodes into the 4 source windows (balancing each destination's
    in-edge window spread against W_CAPS shares), then pack each window's
    nodes into its dest blocks with a worst-dimension-first constructive
    pass.  Returns pos[node]."""
    frac = (W_CAPS / W_CAPS.sum()).astype(np.float32)
    w_slots = np.full(NW, WB_GLOBAL * P)
    caps_nodes = w_slots - 56  # keep free slots (zero rows) in every window

    deg_in = np.bincount(col, minlength=N_NODES).astype(np.int64)

    # out-edge CSR (real edges only; reference self-loops handled separately)
    o = np.argsort(row, kind="stable")
    sr = row[o]
    sc = col[o]
    starts = np.searchsorted(sr, np.arange(N_NODES + 1))
    deg_out = np.diff(starts)
    target = deg_in.astype(np.float32)[:, None] * frac[None, :]

    color = np.full(N_NODES, -1, np.int8)
    kmat = np.zeros((N_NODES, NW), np.int32)
    sizes = np.zeros(NW, np.int64)
    # heavy sources first: they get spread by target deviation while all
    # windows are open; the light tail then fills remaining capacity.
    order_src = rng.permutation(N_NODES)
    order_src = order_src[np.argsort(-deg_out[order_src], kind="stable")]
    B = 1000
    for i in range(0, N_NODES, B):
        batch = order_src[i:i + B]
        reps = deg_out[batch]
        idx = np.concatenate(
            [np.arange(starts[s], starts[s + 1]) for s in batch]
        )
        dsts = sc[idx]
        srcrep = np.repeat(np.arange(len(batch)), reps)
        dev = kmat[dsts].astype(np.float32) - target[dsts]
        score = np.zeros((len(batch), NW), np.float32)
        np.add.at(score, srcrep, dev)
        score += (sizes / caps_nodes).astype(np.float32) * 0.5 * reps[:, None]
        score[:, sizes >= caps_nodes] = 1e18
        ch = score.argmin(1).astype(np.int8)
        color[batch] = ch
        np.add.at(sizes, ch, 1)
        np.add.at(kmat, (dsts, ch[srcrep]), 1)

    # hard-clamp: batched greedy can overshoot caps; move the lightest
    # nodes out of overfull windows into the ones with most slack
    for w in range(NW):
        excess = int(sizes[w] - caps_nodes[w])
        if excess <= 0:
            continue
        cand = np.where(color == w)[0]
        cand = cand[np.argsort(deg_out[cand], kind="stable")][:excess]
        for n in cand:
            w2 = int(np.argmax(caps_nodes - sizes))
            color[n] = w2
            sizes[w] -= 1
            sizes[w2] += 1
            dsts_n = sc[starts[n]:starts[n + 1]]
            np.add.at(kmat, (dsts_n, w), -1)
            np.add.at(kmat, (dsts_n, w2), 1)

    pos = np.empty(N_NODES, np.int64)
    tgt = W_CAPS[None, :]
    for w in range(NW):
        nodes_w = np.where(color == w)[0]
        kk = kmat[nodes_w].astype(np.float32)
        nb = WB_GLOBAL
        # worst-dimension-first constructive packing: place heavy nodes
        # first, each into the block whose post-insert worst dimension
        # (relative to the per-window caps) is smallest.
        order_n = np.argsort(-(kk / tgt).max(1), kind="stable")
        cnt_b = np.zeros((nb, NW), np.float32)
        nslot = np.zeros(nb, np.int64)
        blk = np.empty(len(nodes_w), np.int64)
        slot_of = np.empty(len(nodes_w), np.int64)
        for j in order_n:
            score = ((cnt_b + kk[j][None, :]) / tgt).max(1)
            score[nslot >= P] = 1e9
            b = int(np.argmin(score))
            blk[j] = b
            slot_of[j] = nslot[b]
            nslot[b] += 1
            cnt_b[b] += kk[j]
        # repair: swap nodes between blocks until no (block, dim) exceeds
        # its cap (or no improving swap exists)
        T = W_CAPS
        for _pass in range(10):
            viol = np.argwhere(cnt_b > T[None, :])
            if len(viol) == 0:
                break
            improved = False
            for b, wd in viol:
                if cnt_b[b, wd] <= T[wd]:
                    continue
                members = np.where(blk == b)[0]
                dn = members[np.argsort(-kk[members, wd], kind="stable")][:8]
                rb = np.argsort(cnt_b[:, wd], kind="stable")[:16]
                done_bw = False
                for b2 in rb:
                    if b2 == b or done_bw:
                        continue
                    m2 = np.where(blk == b2)[0]
                    up = m2[np.argsort(kk[m2, wd], kind="stable")][:8]
                    for n1 in dn:
                        k1 = kk[n1]
                        ok = False
                        for n2 in up:
                            k2 = kk[n2]
                            if k2[wd] >= k1[wd]:
                                break
                            nb_ = cnt_b[b] - k1 + k2
                            nb2 = cnt_b[b2] + k1 - k2
                            if (nb_ <= np.maximum(cnt_b[b], T)).all() and (
                                    nb2 <= T).all():
                                blk[n1], blk[n2] = b2, b
                                slot_of[n1], slot_of[n2] = slot_of[n2], slot_of[n1]
                                cnt_b[b] = nb_
                                cnt_b[b2] = nb2
                                improved = True
                                ok = True
                                break
                        if ok and cnt_b[b, wd] <= T[wd]:
                            done_bw = True
                            break
            if not improved:
                break
        pos[nodes_w] = (w * WB_GLOBAL + blk) * P + slot_of
    return pos


def _preprocess(x, edge_index, W1, b1, W2, b2, Wfc, bfc):
    """Host-side graph preprocessing. Returns (plan, in_maps, pos)."""
    row = np.asarray(edge_index[0], dtype=np.int64)
    col = np.asarray(edge_index[1], dtype=np.int64)

    deg = np.bincount(col, minlength=N_NODES).astype(np.int64) + 1
    dinv = (1.0 / np.sqrt(deg.astype(np.float32))).astype(np.float32)

    rng = np.random.default_rng(12345)
    pos = _assign_positions(row, col, rng)

    # zero (pad) rows per window
    occupied = np.zeros(NPOS, bool)
    occupied[pos] = True
    zrow_local = np.empty(NW, np.int64)
    for w in range(NW):
        free = np.where(~occupied[w * W_SPAN:(w + 1) * W_SPAN])[0]
        assert len(free) > 0
        zrow_local[w] = free[-1]

    # --- edge lists (real edges only) in position space
    spos = pos[row]
    dpos = pos[col]
    w_e = spos // W_SPAN
    lidx = spos % W_SPAN
    gb = dpos // P
    slot = (dpos % P).astype(np.float32)
    core_e = gb // BLOCKS
    lb_e = gb % BLOCKS
    dinv_e = dinv[col]

    # counts per (core, lb, w) and the shared round plan
    key_full = (core_e * BLOCKS + lb_e) * NW + w_e
    cnt = np.bincount(key_full, minlength=N_CORES * BLOCKS * NW).reshape(
        N_CORES, BLOCKS, NW
    )
    Rbw = -(-cnt.max(axis=0) // P)  # [BLOCKS, NW] rounds (ceil)
    real_edges = int(cnt.sum())

    # global round order: window-major (all of window 0's rounds in block
    # order, then window 1, ...) so 8-round gather calls pack across block
    # boundaries with zero waste
    wRtot = Rbw.sum(axis=0)                      # rounds per window stream
    wstart = np.concatenate([[0], np.cumsum(wRtot)])
    lbase = np.zeros((NW, BLOCKS), np.int64)     # block offset within stream
    for w in range(NW):
        lbase[w, 1:] = np.cumsum(Rbw[:-1, w])
    rbase = np.zeros((BLOCKS, NW), np.int64)
    for lb in range(BLOCKS):
        for w in range(NW):
            rbase[lb, w] = wstart[w] + lbase[w, lb]
    R_total = int(wstart[NW])
    inflation = R_total * P * N_CORES / real_edges

    # call plan: per window stream, chunks of MAX_CALL_ROUNDS rounds
    # entries: (r0_global, rounds, i8_off); idx8 stream is call-ordered
    calls = []
    win_calls = [[] for _ in range(NW)]
    idx8_off = 0
    for w in range(NW):
        q = 0
        while q < int(wRtot[w]):
            ch = min(MAX_CALL_ROUNDS, int(wRtot[w]) - q)
            c_rec = (int(wstart[w]) + q, ch, idx8_off)
            calls.append((w, int(wstart[w]) + q, ch, idx8_off))
            win_calls[w].append(c_rec)
            idx8_off += ch * 8
            q += ch
    idx8_total = idx8_off
    # idx-chunk loads: per window, IDXCALLS consecutive calls per chunk
    win_ichunks = []
    for w in range(NW):
        chunks = []
        for c0 in range(0, len(win_calls[w]), IDXCALLS):
            sub = win_calls[w][c0:c0 + IDXCALLS]
            i8s = sub[0][2]
            i8e = sub[-1][2] + sub[-1][1] * 8
            chunks.append((i8s, i8e - i8s))
        win_ichunks.append(chunks)

    # window of each global round (for pad-index defaults)
    wor = np.empty(R_total, np.int64)
    for w in range(NW):
        wor[wstart[w]:wstart[w + 1]] = w

    # --- per-core gather metadata
    order = np.lexsort((w_e, lb_e, core_e))
    so_core = core_e[order]
    so_key = (lb_e[order] * NW + w_e[order]).astype(np.int64)
    so_lidx = lidx[order]
    so_slot = slot[order]
    so_dinv = dinv_e[order]
    core_starts = np.searchsorted(so_core, np.arange(N_CORES + 1))

    rbase_flat = rbase.ravel()
    in_maps = []
    xb = np.ascontiguousarray(x).astype(BF16)
    w1_in = np.ascontiguousarray(
        np.asarray(W1).astype(BF16).reshape(4, P, HID).transpose(1, 0, 2).reshape(P, 4 * HID)
    )
    w2_in = np.ascontiguousarray(
        np.asarray(W2).astype(BF16).reshape(2, P, HID).transpose(1, 0, 2).reshape(P, 2 * HID)
    )
    wfc_in = np.ascontiguousarray(
        np.asarray(Wfc).astype(BF16).reshape(2, P, NCLS).transpose(1, 0, 2).reshape(P, 2 * NCLS)
    )
    iota_in = np.tile(np.arange(P, dtype=np.float32).astype(BF16)[None, :], (P, 1))
    iotap_in = np.arange(P, dtype=np.float32).reshape(P, 1).copy()
    has_b1 = bool(np.any(b1)); has_b2 = bool(np.any(b2)); has_bfc = bool(np.any(bfc))
    b1_in = np.asarray(b1, np.float32).reshape(2, P).T.copy()   # [128, 2]
    b2_in = np.asarray(b2, np.float32).reshape(2, P).T.copy()
    bfc_in = np.tile(np.asarray(bfc, np.float32).astype(BF16)[None, :], (P, 1))

    pad_zrow = zrow_local[wor]  # [R_total] default local idx per round

    for c in range(N_CORES):
        s0, s1 = core_starts[c], core_starts[c + 1]
        keys_c = so_key[s0:s1]
        # j-within-group for each edge (groups are contiguous, sorted)
        grp_start = np.searchsorted(keys_c, np.arange(BLOCKS * NW))
        jw = np.arange(s1 - s0) - grp_start[keys_c]
        ecol = rbase_flat[keys_c] + jw // P
        erow = jw % P

        destid = np.zeros((P, R_total), np.float32)
        dinvdst = np.ones((P, R_total), np.float32)
        destid[erow, ecol] = so_slot[s0:s1]
        dinvdst[erow, ecol] = so_dinv[s0:s1]

        # flat (round-major) local indices, defaulting to the window's 0-row
        flat_idx = np.repeat(pad_zrow, P).astype(np.int64)
        flat_idx[ecol * P + erow] = so_lidx[s0:s1]
        assert flat_idx.max() < 32768
        idxs2d = np.empty((P, idx8_total), np.int16)
        for (w, r0, ch, i8) in calls:
            seg = flat_idx[r0 * P:(r0 + ch) * P].astype(np.int16)
            idxs2d[:, i8:i8 + ch * 8] = _wrap_idx(seg)

        # dense per-core inputs
        ids_c = np.where((pos >= c * SLOTS) & (pos < (c + 1) * SLOTS))[0]
        lpos = pos[ids_c] - c * SLOTS
        A = np.zeros((SLOTS, IN_DIM), BF16)
        A[lpos] = xb[ids_c]
        xtt = np.ascontiguousarray(
            A.reshape(BLOCKS, P, 4, P).transpose(0, 3, 2, 1).reshape(BLOCKS, P, IN_DIM)
        )
        dv = np.ones(SLOTS, np.float32)
        dv[lpos] = dinv[ids_c]
        dvp = np.ascontiguousarray(dv.reshape(BLOCKS, P).T)  # [128, 98]

        m = {
            "xtt": xtt,
            "dinvp": dvp,
            "idxs": idxs2d,
            "destid": destid,
            "dinvdst": dinvdst,
            "iota": iota_in,
            "iotap": iotap_in,
            "w1": w1_in,
            "w2": w2_in,
            "wfc": wfc_in,
        }
        if has_b1:
            m["b1h"] = b1_in
        if has_b2:
            m["b2h"] = b2_in
        if has_bfc:
            m["bfcb"] = bfc_in
        in_maps.append(m)

    plan = {
        "win_calls": win_calls,
        "win_ichunks": win_ichunks,
        "lbase": lbase.tolist(),
        "wstart": wstart.tolist(),
        "rbase": rbase.tolist(),
        "Rbw": Rbw.tolist(),
        "R_total": R_total,
        "idx8_total": idx8_total,
        "n_calls": len(calls),
        "has_b1": has_b1,
        "has_b2": has_b2,
        "has_bfc": has_bfc,
        "inflation": inflation,
    }
    return plan, in_maps, pos


def _build_program(plan, sim_single_core=False, stop_after="full"):
    """Build the SPMD Bass program (one program, all cores).

    stop_after: one of "mm1", "ag1", "g1", "ag2", "g2", "full" — truncates
    the program after that phase (for bisection/debug)."""
    STAGES = ["mm1", "ag1", "g1", "ag2", "g2", "full"]
    stop_idx = STAGES.index(stop_after)
    nc = bacc.Bacc("TRN2", target_bir_lowering=False, debug=False,
                   num_devices=N_CORES)
    dt = mybir.dt

    R_total = plan["R_total"]
    idx8_total = plan["idx8_total"]
    win_calls = plan["win_calls"]
    win_ichunks = plan["win_ichunks"]
    lbase = plan["lbase"]
    wstart = plan["wstart"]
    Rbw = plan["Rbw"]

    xtt = nc.dram_tensor("xtt", [BLOCKS, P, IN_DIM], dt.bfloat16, kind="ExternalInput")
    dinvp = nc.dram_tensor("dinvp", [P, BLOCKS], dt.float32, kind="ExternalInput")
    idxs = nc.dram_tensor("idxs", [P, idx8_total], dt.int16, kind="ExternalInput")
    destid = nc.dram_tensor("destid", [P, R_total], dt.float32, kind="ExternalInput")
    dinvdst = nc.dram_tensor("dinvdst", [P, R_total], dt.float32, kind="ExternalInput")
    iota = nc.dram_tensor("iota", [P, P], dt.bfloat16, kind="ExternalInput")
    iotap = nc.dram_tensor("iotap", [P, 1], dt.float32, kind="ExternalInput")
    w1 = nc.dram_tensor("w1", [P, 4 * HID], dt.bfloat16, kind="ExternalInput")
    w2 = nc.dram_tensor("w2", [P, 2 * HID], dt.bfloat16, kind="ExternalInput")
    wfc = nc.dram_tensor("wfc", [P, 2 * NCLS], dt.bfloat16, kind="ExternalInput")
    b1h = (nc.dram_tensor("b1h", [P, 2], dt.float32, kind="ExternalInput")
           if plan["has_b1"] else None)
    b2h = (nc.dram_tensor("b2h", [P, 2], dt.float32, kind="ExternalInput")
           if plan["has_b2"] else None)
    bfcb = (nc.dram_tensor("bfcb", [P, NCLS], dt.bfloat16, kind="ExternalInput")
            if plan["has_bfc"] else None)
    out = nc.dram_tensor("out", [SLOTS, NCLS], dt.bfloat16, kind="ExternalOutput")

    hloc1 = nc.dram_tensor("hloc1", [SLOTS, HID], dt.bfloat16)
    hloc2 = nc.dram_tensor("hloc2", [SLOTS, HID], dt.bfloat16)
    hfull1 = nc.dram_tensor("hfull1", [NPOS, HID], dt.bfloat16, addr_space="Shared")
    hfull2 = nc.dram_tensor("hfull2", [NPOS, HID], dt.bfloat16, addr_space="Shared")

    RELU = mybir.ActivationFunctionType.Relu
    COPY = mybir.ActivationFunctionType.Copy

    with tile.TileContext(nc) as tc:
        with (
            tc.tile_pool(name="const", bufs=1) as constp,
            tc.tile_pool(name="xt", bufs=4) as xtp,
            tc.tile_pool(name="hla", bufs=1) as hlap,
            tc.tile_pool(name="idx", bufs=2) as idxp,
            tc.tile_pool(name="g", bufs=4) as gp,
            tc.tile_pool(name="m", bufs=8) as mp,
            tc.tile_pool(name="at", bufs=4) as atp,
            tc.tile_pool(name="fco", bufs=2) as fcop,
            tc.tile_pool(name="mmps", bufs=2, space="PSUM") as mmps,
            tc.tile_pool(name="aggps", bufs=2, space="PSUM") as aggps,
            tc.tile_pool(name="fcps", bufs=1, space="PSUM") as fcps,
        ):
            # constants needed by the layer-1 matmul, loaded up front
            w1_sb = constp.tile([P, 4 * HID], dt.bfloat16)
            nc.sync.dma_start(out=w1_sb[:], in_=w1[:])
            dv_sb = constp.tile([P, BLOCKS], dt.float32)
            nc.sync.dma_start(out=dv_sb[:], in_=dinvp[:])

            def load_agg_consts():
                """Aggregation-phase constants; emitted after the layer-1
                matmul DMAs so they don't delay the first x loads."""
                w2_sb = constp.tile([P, 2 * HID], dt.bfloat16)
                nc.sync.dma_start(out=w2_sb[:], in_=w2[:])
                wfc_sb = constp.tile([P, 2 * NCLS], dt.bfloat16)
                nc.sync.dma_start(out=wfc_sb[:], in_=wfc[:])
                iota_sb = constp.tile([P, P], dt.bfloat16)
                nc.sync.dma_start(out=iota_sb[:], in_=iota[:])
                iotap_sb = constp.tile([P, 1], dt.float32)
                nc.sync.dma_start(out=iotap_sb[:], in_=iotap[:])
                dst_sb = constp.tile([P, R_total], dt.float32)
                nc.sync.dma_start(out=dst_sb[:], in_=destid[:])
                dnv_sb = constp.tile([P, R_total], dt.float32)
                nc.sync.dma_start(out=dnv_sb[:], in_=dinvdst[:])
                b1_sb = b2_sb = bfc_sb = None
                if b1h is not None:
                    b1_sb = constp.tile([P, 2], dt.float32)
                    nc.sync.dma_start(out=b1_sb[:], in_=b1h[:])
                if b2h is not None:
                    b2_sb = constp.tile([P, 2], dt.float32)
                    nc.sync.dma_start(out=b2_sb[:], in_=b2h[:])
                if bfcb is not None:
                    bfc_sb = constp.tile([P, NCLS], dt.bfloat16)
                    nc.sync.dma_start(out=bfc_sb[:], in_=bfcb[:])
                return w2_sb, wfc_sb, iota_sb, iotap_sb, dst_sb, dnv_sb, \
                    b1_sb, b2_sb, bfc_sb

            # resident per-block feature tiles (one big buffer, slice per
            # block); rewritten in place by layer 2
            hl_all = hlap.tile([P, BLOCKS * HID], dt.bfloat16)

            def all_gather(hloc, hfull):
                if sim_single_core:
                    nc.sync.dma_start(out=hfull[0:SLOTS, :], in_=hloc[:])
                else:
                    nc.gpsimd.collective_compute(
                        "AllGather",
                        mybir.AluOpType.bypass,
                        replica_groups=[list(range(N_CORES))],
                        ins=[hloc[:]],
                        outs=[hfull[:]],
                    )

            def hloc_flush(hloc, b0, nb):
                """One batched DMA writing blocks [b0, b0+nb) of hl_all."""
                nc.scalar.dma_start(
                    out=hloc[b0 * P:(b0 + nb) * P, :].rearrange(
                        "(b p) d -> p b d", p=P),
                    in_=hl_all[:, b0 * HID:(b0 + nb) * HID].rearrange(
                        "p (b d) -> p b d", d=HID),
                )

            def new_stream_state():
                return {
                    "next": [0] * NW,          # next call index per window
                    "gt": [dict() for _ in range(NW)],   # call -> gather tile
                    "ich": [dict() for _ in range(NW)],  # chunk -> (tile, i8s)
                }

            def issue_call(st, w, hfull):
                c = st["next"][w]
                r0, ch, i8 = win_calls[w][c]
                ck = c // IDXCALLS
                for ckx in (ck, ck + 1):
                    if (ckx < len(win_ichunks[w]) and ckx not in st["ich"][w]
                            and (ckx == ck or c % IDXCALLS >= IDXCALLS // 2)):
                        ci8s, ci8n = win_ichunks[w][ckx]
                        it = idxp.tile([P, ci8n], dt.int16, tag=f"i{w}",
                                       padded_shape=[P, IDXCALLS * 8 * MAX_CALL_ROUNDS],
                                       name=f"it{w}")
                        nc.sync.dma_start(out=it[:], in_=idxs[:, ci8s:ci8s + ci8n])
                        st["ich"][w][ckx] = (it, ci8s)
                it, ci8s = st["ich"][w][ck]
                gt = gp.tile([P, ch * HID], dt.bfloat16, tag=f"g{w}",
                             padded_shape=[P, MAX_CALL_ROUNDS * HID],
                             name=f"gt{w}")
                nc.gpsimd.dma_gather(
                    gt[:, :ch * HID].rearrange("p (l d) -> p l d", d=HID),
                    hfull[w * W_SPAN:(w + 1) * W_SPAN, :],
                    it[:, i8 - ci8s:i8 - ci8s + ch * 8],
                    ch * P,
                    ch * P,
                    HID,
                )
                st["gt"][w][c] = gt
                st["next"][w] = c + 1

            def ensure_calls(st, lb, hfull):
                """Issue every call needed through block lb (lookahead)."""
                for w in range(NW):
                    last = (lbase[w][lb] + Rbw[lb][w] - 1) // MAX_CALL_ROUNDS
                    while st["next"][w] <= last:
                        issue_call(st, w, hfull)

            def agg_block(st, lb, hfull, b_sb):
                """One-hot matmul aggregation for dest block lb, consuming
                the window-major gather streams.  Returns the aggT sbuf tile
                [128, 256] bf16 (partition = feat%128, free = dest slot),
                after relu."""
                ensure_calls(st, min(lb + 1, BLOCKS - 1), hfull)
                ps0 = aggps.tile([P, P], dt.float32, space="PSUM", tag="agg0")
                ps1 = aggps.tile([P, P], dt.float32, space="PSUM", tag="agg1")
                # self-loop contribution from the resident feature slice
                D = mp.tile([P, P], dt.bfloat16, tag="m", name="D")
                nc.vector.tensor_scalar(
                    out=D[:], in0=iota_sb[:],
                    scalar1=iotap_sb[:, 0:1],
                    scalar2=dv_sb[:, lb:lb + 1],
                    op0=mybir.AluOpType.is_equal,
                    op1=mybir.AluOpType.mult,
                )
                hlt = hl_all[:, lb * HID:(lb + 1) * HID]
                n_rounds = sum(Rbw[lb])
                for h in range(2):
                    nc.tensor.matmul(
                        out=(ps0 if h == 0 else ps1)[:],
                        lhsT=hlt[:, h * P:h * P + P],
                        rhs=D[:],
                        start=True,
                        stop=False,
                    )
                done = 0
                for w in range(NW):
                    for r in range(Rbw[lb][w]):
                        q = lbase[w][lb] + r
                        rc = wstart[w] + q
                        c = q // MAX_CALL_ROUNDS
                        off = q - c * MAX_CALL_ROUNDS
                        gt = st["gt"][w][c]
                        M = mp.tile([P, P], dt.bfloat16, tag="m", name="M")
                        nc.vector.tensor_scalar(
                            out=M[:], in0=iota_sb[:],
                            scalar1=dst_sb[:, rc:rc + 1],
                            scalar2=dnv_sb[:, rc:rc + 1],
                            op0=mybir.AluOpType.is_equal,
                            op1=mybir.AluOpType.mult,
                        )
                        for h in range(2):
                            nc.tensor.matmul(
                                out=(ps0 if h == 0 else ps1)[:],
                                lhsT=gt[:, off * HID + h * P:off * HID + h * P + P],
                                rhs=M[:],
                                start=False,
                                stop=(done == n_rounds - 1),
                            )
                        done += 1
                at = atp.tile([P, HID], dt.bfloat16, tag="at", name="at")
                for h in range(2):
                    nc.scalar.activation(
                        out=at[:, h * P:(h + 1) * P],
                        in_=(ps0 if h == 0 else ps1)[:],
                        func=RELU,
                        bias=(b_sb[:, h:h + 1] if b_sb is not None else 0.0),
                    )
                return at

            # ---- layer 1: h1' = dinv * (x @ W1); results stay resident.
            # x loaded 4 blocks per DMA; PSUM tiles hold 2 blocks (a full
            # bank) so the PE stays continuously busy and ramps to full clock
            for mb in range(0, BLOCKS, 4):
                nb4 = min(4, BLOCKS - mb)
                at = xtp.tile([P, 4 * IN_DIM], dt.bfloat16, tag="xt", name="at")
                nc.sync.dma_start(
                    out=at[:, :nb4 * IN_DIM].rearrange("p (b d) -> p b d", d=IN_DIM),
                    in_=xtt[mb:mb + nb4].rearrange("b p d -> p b d"),
                )
                for j0 in range(0, nb4, 2):
                    nj = min(2, nb4 - j0)
                    sel = ((mb + j0) // 2) % 4
                    if sel < 2:
                        ps = mmps.tile([P, 2 * HID], dt.float32, space="PSUM",
                                       tag="mm")
                    else:
                        ps = fcps.tile([P, 2 * HID], dt.float32, space="PSUM",
                                       tag=f"fc{sel - 2}", name="ps")
                    for j in range(j0, j0 + nj):
                        for k in range(4):
                            nc.tensor.matmul(
                                out=ps[:, (j - j0) * HID:(j - j0 + 1) * HID],
                                lhsT=at[:, j * IN_DIM + k * P:j * IN_DIM + (k + 1) * P],
                                rhs=w1_sb[:, k * HID:(k + 1) * HID],
                                start=(k == 0),
                                stop=(k == 3),
                            )
                    for j in range(j0, j0 + nj):
                        nc.vector.tensor_scalar(
                            out=hl_all[:, (mb + j) * HID:(mb + j + 1) * HID],
                            in0=ps[:, (j - j0) * HID:(j - j0 + 1) * HID],
                            scalar1=dv_sb[:, mb + j:mb + j + 1],
                            scalar2=None,
                            op0=mybir.AluOpType.mult,
                        )
            (w2_sb, wfc_sb, iota_sb, iotap_sb, dst_sb, dnv_sb,
             b1_sb, b2_sb, bfc_sb) = load_agg_consts()
            for b0 in range(0, BLOCKS, HLOC_BATCH):
                hloc_flush(hloc1, b0, min(HLOC_BATCH, BLOCKS - b0))
            if stop_idx >= 1:
                all_gather(hloc1, hfull1)
            # ---- layer 1 aggregation + layer 2 matmul, fused per block
            st1 = new_stream_state()
            for lb in range(BLOCKS if stop_idx >= 2 else 0):
                at1 = agg_block(st1, lb, hfull1, b1_sb)
                ps2 = mmps.tile([P, HID], dt.float32, space="PSUM", tag="mm")
                for k in range(2):
                    nc.tensor.matmul(
                        out=ps2[:],
                        lhsT=at1[:, k * P:(k + 1) * P],
                        rhs=w2_sb[:, k * HID:(k + 1) * HID],
                        start=(k == 0),
                        stop=(k == 1),
                    )
                nc.scalar.activation(
                    out=hl_all[:, lb * HID:(lb + 1) * HID],
                    in_=ps2[:], func=COPY, scale=dv_sb[:, lb:lb + 1],
                )
                if lb % HLOC_BATCH == HLOC_BATCH - 1 or lb == BLOCKS - 1:
                    b0 = (lb // HLOC_BATCH) * HLOC_BATCH
                    hloc_flush(hloc2, b0, lb - b0 + 1)
            if stop_idx >= 3:
                all_gather(hloc2, hfull2)
            # ---- layer 2 aggregation + FC head, fused per block
            st2 = new_stream_state()
            fo = None
            for lb in range(BLOCKS if stop_idx >= 4 else 0):
                at2 = agg_block(st2, lb, hfull2, b2_sb)
                if lb % 2 == 0:
                    fo = fcop.tile([P, 2 * NCLS], dt.bfloat16, tag="fco",
                                   name="fo")
                half = (lb % 2) * NCLS
                for n in range(2):
                    ps = fcps.tile([P, NCLS // 2], dt.float32, space="PSUM",
                                   tag=f"fc{n}")
                    for k in range(2):
                        nc.tensor.matmul(
                            out=ps[:],
                            lhsT=at2[:, k * P:(k + 1) * P],
                            rhs=wfc_sb[:, k * NCLS + n * (NCLS // 2):
                                       k * NCLS + (n + 1) * (NCLS // 2)],
                            start=(k == 0),
                            stop=(k == 1),
                        )
                    nc.scalar.activation(
                        out=fo[:, half + n * (NCLS // 2):
                               half + (n + 1) * (NCLS // 2)],
                        in_=ps[:], func=COPY,
                    )
                if bfc_sb is not None:
                    nc.vector.tensor_tensor(
                        out=fo[:, half:half + NCLS],
                        in0=fo[:, half:half + NCLS], in1=bfc_sb[:],
                        op=mybir.AluOpType.add,
                    )
                if stop_idx >= 5 and (lb % 2 == 1 or lb == BLOCKS - 1):
                    lb0 = lb - lb % 2
                    nc.scalar.dma_start(
                        out=out[lb0 * P:(lb + 1) * P, :].rearrange(
                            "(b p) d -> p b d", p=P),
                        in_=fo[:, :(lb - lb0 + 1) * NCLS].rearrange(
                            "p (b d) -> p b d", d=NCLS),
                    )

    nc.compile()
    return nc


def kernel(x, edge_index, W1, b1, W2, b2, Wfc, bfc):
    x = np.asarray(x)
    plan, in_maps, pos = _preprocess(x, edge_index, W1, b1, W2, b2, Wfc, bfc)
    nc = _build_program(plan)
    res = run_bass_kernel_spmd(nc, in_maps, core_ids=list(range(N_CORES)))
    full = np.empty((N_NODES, NCLS), np.float32)
    core = pos // SLOTS
    lrow = pos % SLOTS
    for c in range(N_CORES):
        sel = core == c
        full[sel] = res.results[c]["out"][lrow[sel]].astype(np.float32)
    return full
